# revision 3
# baseline (speedup 1.0000x reference)
"""Causal self-attention (RoPE) Trainium2 kernel, 8-core SPMD. v4.

Sharding: core c -> (batch b = c//2, head-group g = c%2). Each core computes
its batch's attention for its 8 heads, applies its 512 rows of Wo^T, and the
host sums the two head-group partials per batch (Megatron row-parallel).

v4 vs v3: the two heads sharing a 128-partition tile (hb=0 / hb=64) are
processed TOGETHER per (ot, chunk) unit. Their K=64 score matmuls are
emitted back-to-back so the PE runs them concurrently as (0,0)/(64,0)
row-tiles (tile_position auto-derived from base partitions; ~2x on the
score phase on HW - the cost model serializes them, hardware does not).
Score PSUM is one shared [128, 1024] tile (A cols 0:512, B 512:1024) so a
single wide 3D-AP exp covers both heads, halving activation fixed costs.
s1 is walked in 512-col chunks (4 per head); PSUM: 2x2 banks scores,
2x1 banks PV accumulators, 2 banks projection/rope accumulators.
"""

import math

import numpy as np
import ml_dtypes

BF16 = ml_dtypes.bfloat16

B, S, DIM = 4, 2048, 1024
NUM_HEADS = 16
HEAD_DIM = 64
ROPE_BASE = 10000.0
N_CORES = 8
HG = 8          # heads per core (head-group)
O = HG * HEAD_DIM  # 512 per-core projection width
CW = 512        # attention s1 chunk width

_NC = None  # cached compiled Bass program

# feature flags (HW-bringup bisection)
WIDE_EXP = True     # one 3D-AP exp covering both heads vs two 2D exps
EARLY_NORM = False  # early-norm cols [0:256] of unit (3,3) (split PV groups)


def _rope_tables():
    inv_freq = 1.0 / (ROPE_BASE ** (np.arange(0, HEAD_DIM, 2, dtype=np.float64) / HEAD_DIM))
    t = np.arange(S, dtype=np.float64)
    freqs = np.einsum("i,j->ij", t, inv_freq)          # (S, 32)
    emb = np.concatenate([freqs, freqs], axis=-1)      # (S, 64)
    cos = np.cos(emb).astype(np.float32)
    sin = np.sin(emb).astype(np.float32)
    # transposed + tiled to 128 partitions (2 heads per 128-row tile)
    cosT = np.tile(cos.T, (2, 1))                      # (128, S)
    sinT = np.tile(sin.T, (2, 1))
    return cosT, sinT


def _rot_matrix():
    # rotate_half as a matrix: out[d] = -q[d+32] (d<32), q[d-32] (d>=32)
    r = np.zeros((HEAD_DIM, HEAD_DIM), dtype=np.float32)
    for d in range(32):
        r[d, d + 32] = -1.0
        r[d + 32, d] = 1.0
    r128 = np.zeros((128, 128), dtype=np.float32)
    r128[:64, :64] = r
    r128[64:, 64:] = r
    return r128.T.copy()  # lhsT for out = R @ q


def _build_nc(reps=1):
    from contextlib import ExitStack

    import concourse.tile as tile
    from concourse import bacc
    import concourse.mybir as mybir

    f32 = mybir.dt.float32
    bf16 = mybir.dt.bfloat16

    nc = bacc.Bacc("TRN2", target_bir_lowering=False, debug=False,
                   num_devices=N_CORES)

    # host-retiled layouts: one contiguous 8KB run per partition per DMA
    xT = nc.declare_dram_parameter("xTt", [4 * 128, 4096], bf16,
                                   isOutput=False)
    wqT = nc.declare_dram_parameter("wqTt", [128, 4096], bf16, isOutput=False)
    wkT = nc.declare_dram_parameter("wkTt", [128, 4096], bf16, isOutput=False)
    wvT = nc.declare_dram_parameter("wvTt", [128, 4096], bf16, isOutput=False)
    woT = nc.declare_dram_parameter("woTt", [128, 4096], bf16, isOutput=False)
    cosT = nc.declare_dram_parameter("cosT", [128, S], bf16, isOutput=False)
    sinT = nc.declare_dram_parameter("sinT", [128, S], bf16, isOutput=False)
    rT = nc.declare_dram_parameter("rT", [128, 128], bf16, isOutput=False)
    dmask = nc.declare_dram_parameter("dmask", [128, 128], bf16, isOutput=False)
    out = nc.declare_dram_parameter("out", [S, DIM], bf16, isOutput=True)

    with tile.TileContext(nc) as tc, ExitStack() as top:
        for _ in range(reps):
            _emit_body(nc, tc, mybir, xT, wqT, wkT, wvT, woT,
                       cosT, sinT, rT, dmask, out)

    nc.compile()
    return nc


def _emit_body(nc, tc, mybir, xT, wqT, wkT, wvT, woT,
               cosT, sinT, rT, dmask, out):
    from contextlib import ExitStack

    f32 = mybir.dt.float32
    bf16 = mybir.dt.bfloat16

    xT3 = xT.ap().rearrange("(c p) f -> p c f", p=128)      # (128, 4, 4096)

    with ExitStack() as top:
        otp = top.enter_context(tc.tile_pool(name="otp", bufs=1))
        qk = top.enter_context(tc.tile_pool(name="qk", bufs=1))
        cst = top.enter_context(tc.tile_pool(name="cst", bufs=1))
        xp = top.enter_context(tc.tile_pool(name="xp", bufs=4))
        wp = top.enter_context(tc.tile_pool(name="wp", bufs=3))
        wop = top.enter_context(tc.tile_pool(name="wop", bufs=1))
        tp = top.enter_context(tc.tile_pool(name="tp", bufs=4))
        ep = top.enter_context(tc.tile_pool(name="ep", bufs=8))
        rp = top.enter_context(tc.tile_pool(name="rp", bufs=4))
        bp = top.enter_context(tc.tile_pool(name="bp", bufs=4))
        stg = top.enter_context(tc.tile_pool(name="stg", bufs=4))

        OT = otp.tile([128, 4, S], bf16)   # normalized attn out, transposed
        QT = qk.tile([128, 4, S], bf16)
        KT = qk.tile([128, 4, S], bf16)
        VA = qk.tile([128, 16, 520], bf16)  # [V(64) | ones] per head

        cos_sb = cst.tile([128, S], bf16)
        sin_sb = cst.tile([128, S], bf16)
        rt_sb = cst.tile([128, 128], bf16)
        dm_sb = cst.tile([128, 128], bf16)
        wo_sb = wop.tile([128, 4096], bf16)

        # ---------------- DMA issue (order = priority) ----------------
        # wq/wk are host-retiled ot-major: cols [ot*1024:(ot+1)*1024] hold
        # pair ot's 8 kt-chunks of 128, so the first unit's weights are one
        # contiguous quarter. x chunks are kt-major (any emit reads it all);
        # quarter-split x0 so the prologue's kt-sequential reads start early.
        wk_sb = wp.tile([128, 4096], bf16, tag="w", name="wk_sb")
        wq_sb = wp.tile([128, 4096], bf16, tag="w", name="wq_sb")
        wv_sb = wp.tile([128, 4096], bf16, tag="w", name="wv_sb")
        xs = [xp.tile([128, 4096], bf16, tag="x", name=f"x{sc}")
              for sc in range(4)]
        nc.sync.dma_start(wk_sb[:, 0:1024], wkT.ap()[:, 0:1024])
        for q in range(4):
            nc.sync.dma_start(xs[0][:, q * 1024:(q + 1) * 1024],
                              xT3[:, 0, q * 1024:(q + 1) * 1024])
        nc.sync.dma_start(wq_sb[:, 0:1024], wqT.ap()[:, 0:1024])
        nc.sync.dma_start(wv_sb[:, 0:2048], wvT.ap()[:, 0:2048])
        nc.sync.dma_start(wv_sb[:, 2048:4096], wvT.ap()[:, 2048:4096])
        nc.sync.dma_start(wk_sb[:, 1024:2048], wkT.ap()[:, 1024:2048])
        nc.sync.dma_start(wq_sb[:, 1024:2048], wqT.ap()[:, 1024:2048])
        nc.sync.dma_start(wk_sb[:, 2048:4096], wkT.ap()[:, 2048:4096])
        nc.sync.dma_start(wq_sb[:, 2048:4096], wqT.ap()[:, 2048:4096])
        nc.sync.dma_start(xs[1][:, 0:2048], xT3[:, 1, 0:2048])
        nc.sync.dma_start(xs[1][:, 2048:4096], xT3[:, 1, 2048:4096])
        for sc in range(2, 4):
            nc.sync.dma_start(xs[sc][:], xT3[:, sc, :])
        nc.gpsimd.dma_start(cos_sb[:, 0:1024], cosT.ap()[:, 0:1024])
        nc.gpsimd.dma_start(sin_sb[:, 0:1024], sinT.ap()[:, 0:1024])
        nc.gpsimd.dma_start(rt_sb[:], rT.ap())
        nc.gpsimd.dma_start(dm_sb[:], dmask.ap())
        nc.gpsimd.dma_start(cos_sb[:, 1024:2048], cosT.ap()[:, 1024:2048])
        nc.gpsimd.dma_start(sin_sb[:, 1024:2048], sinT.ap()[:, 1024:2048])

        # ot-major views for q/k; kt-major full-width for v
        wkt = wk_sb[:].rearrange("p (o k f) -> p o k f", o=4, k=8)
        wqt = wq_sb[:].rearrange("p (o k f) -> p o k f", o=4, k=8)
        wvt = wv_sb[:].rearrange("p (o f) -> p o f", f=512)

        def xtile(sc, kt, csl):
            xv = xs[sc][:].rearrange("p (o s) -> p o s", s=512)
            return xv[:, kt, csl]

        P = {}  # current-scope PSUM pools: "pp", "psc", "pso"

        def rope(acc, dest, ot, sl):
            # dest[:, ot, sl] = acc*cos + R @ (acc*sin)
            rs = tp.tile([128, 512], bf16, tag="t", name="rs")
            nc.vector.tensor_mul(rs[:], acc[:], sin_sb[:, sl])
            rot = P["pp"].tile([128, 512], f32, tag="pp", name="rot")
            nc.tensor.matmul(rot[:], rt_sb[:], rs[:], start=True, stop=True)
            t1 = tp.tile([128, 512], f32, tag="t", name="t1")
            nc.vector.tensor_mul(t1[:], acc[:], cos_sb[:, sl])
            nc.vector.tensor_add(dest[:, ot, sl], t1[:], rot[:])

        def emit_kq(wlist, dest, ot, sc):
            sl = slice(sc * 512, (sc + 1) * 512)
            acc = P["pp"].tile([128, 512], f32, tag="pp", name="acc")
            for kt in range(8):
                nc.tensor.matmul(
                    acc[:],
                    wlist[:, ot, kt, :],
                    xtile(sc, kt, slice(0, 512)),
                    start=(kt == 0), stop=(kt == 7))
            rope(acc, dest, ot, sl)

        def emit_v_one(sc, st):
            s2t = sc * 4 + st
            acc = P["pp"].tile([128, 512], f32, tag="pp", name="acc")
            for kt in range(8):
                nc.tensor.matmul(
                    acc[:],
                    xtile(sc, kt, slice(st * 128, (st + 1) * 128)),
                    wvt[:, kt, :],
                    start=(kt == 0), stop=(kt == 7))
            vsl = VA[:, s2t, :].rearrange("p (h c) -> p h c", c=65)
            nc.scalar.copy(
                vsl[:, :, 0:64],
                acc[:].rearrange("p (h c) -> p h c", c=64))
            nc.gpsimd.memset(vsl[:, :, 64:65], 1.0)

        def emit_attn_pair(ot, cp, fill=None, fill_start=0, early_norm=False):
            # both heads of pair ot; s1 chunk cp (512 wide).
            # fill: list of zero-arg callables, one consumed per j.
            # early_norm: normalize cols [0:256] as soon as their last PV
            # lands (j == 4cp+1) so proj fills of this chunk can run mid-unit.
            njs = 4 * cp + 4
            hA, hB = 2 * ot, 2 * ot + 1
            s1l = cp * CW
            otA = P["pso"].tile([65, CW], f32, tag="oA", name="otA")
            otB = P["pso"].tile([65, CW], f32, tag="oB", name="otB")
            # fill pops are spread every 4th j (end-anchored) and happen
            # BEFORE the pending PV so the fill covers its exp wait;
            # leftovers drain after the norms, covering the next unit's
            # first-PV wait on this unit's pso slots.
            def pops_now(j):
                return (fill and j >= fill_start
                        and (njs - 1 - j) % 4 == 0)

            def emit_pv(j, l0, et):
                # (dst-lo, dst-hi, start, stop) col spans; with early_norm
                # cols [0:256] form their own accumulation group closed at
                # j == 4cp+1 so the early norm may read them mid-unit.
                if early_norm and j <= 4 * cp + 1:
                    # separate spans so cols [0:256] take no further writes
                    # after j == 4cp+1; start=True only on j=0's first span
                    # (it marks the whole 2KB zero region pending-zero), and
                    # skip the per-bank group check since the early norm
                    # reads mid-group (writes are col-disjoint by then).
                    spans = [(l0, 256, j == 0), (256, CW, False)]
                else:
                    spans = [(l0, CW, j == 0)]
                skip = early_norm
                for lo, hi, st in spans:
                    nc.tensor.matmul(
                        otA[:, lo:hi],
                        VA[:, j, hA * 65:(hA + 1) * 65],
                        et[:, lo:hi],
                        start=st, stop=(j == njs - 1),
                        skip_group_check=skip)
                    nc.tensor.matmul(
                        otB[:, lo:hi],
                        VA[:, j, hB * 65:(hB + 1) * 65],
                        et[:, CW + lo:CW + hi],
                        start=st, stop=(j == njs - 1),
                        skip_group_check=skip)

            def norm(otps, hb, lo, hi):
                # GPSIMD cannot touch PSUM, so the otps reads (recip + mul)
                # must stay on DVE; only the broadcast runs on Pool
                w = hi - lo
                rec = rp.tile([1, CW], f32, tag="rec", name="rec")
                nc.vector.reciprocal(rec[:, 0:w], otps[64:65, lo:hi])
                bc = bp.tile([64, CW], f32, tag="bc", name="bc")
                nc.gpsimd.partition_broadcast(bc[:, 0:w], rec[:, 0:w])
                nc.vector.tensor_mul(
                    OT[hb:hb + 64, ot, s1l + lo:s1l + hi],
                    otps[0:64, lo:hi], bc[:, 0:w])

            pending = None
            for j in range(njs):
                l0 = max(0, 128 * j - s1l)
                scp = P["psc"].tile([128, 2 * CW], f32, tag="sc", name="scp")
                nc.tensor.matmul(
                    scp[:, l0:CW],
                    KT[0:64, ot, j * 128:(j + 1) * 128],
                    QT[0:64, ot, s1l + l0:s1l + CW],
                    start=True, stop=True)
                nc.tensor.matmul(
                    scp[:, CW + l0:2 * CW],
                    KT[64:128, ot, j * 128:(j + 1) * 128],
                    QT[64:128, ot, s1l + l0:s1l + CW],
                    start=True, stop=True)
                et = ep.tile([128, 2 * CW], bf16, tag="e", name="et")
                if WIDE_EXP:
                    a_in = scp[:].rearrange("p (b w) -> p b w", b=2)[:, :, l0:CW]
                    a_out = et[:].rearrange("p (b w) -> p b w", b=2)[:, :, l0:CW]
                    nc.scalar.activation(
                        a_out, a_in,
                        mybir.ActivationFunctionType.Exp,
                        scale=1.0 / math.sqrt(HEAD_DIM))
                else:
                    nc.scalar.activation(
                        et[:, l0:CW], scp[:, l0:CW],
                        mybir.ActivationFunctionType.Exp,
                        scale=1.0 / math.sqrt(HEAD_DIM))
                    nc.scalar.activation(
                        et[:, CW + l0:2 * CW], scp[:, CW + l0:2 * CW],
                        mybir.ActivationFunctionType.Exp,
                        scale=1.0 / math.sqrt(HEAD_DIM))
                if 128 * j >= s1l:
                    dl = 128 * j - s1l
                    nc.gpsimd.tensor_mul(
                        et[:, dl:dl + 128], et[:, dl:dl + 128], dm_sb[:])
                    nc.gpsimd.tensor_mul(
                        et[:, CW + dl:CW + dl + 128],
                        et[:, CW + dl:CW + dl + 128], dm_sb[:])
                if pops_now(j):
                    fill.pop(0)()
                if pending is not None:
                    emit_pv(*pending)
                    if early_norm and pending[0] == 4 * cp + 1:
                        norm(otA, 0, 0, 256)
                        norm(otB, 64, 0, 256)
                pending = (j, l0, et)
            emit_pv(*pending)
            lo = 256 if early_norm else 0
            norm(otA, 0, lo, CW)
            norm(otB, 64, lo, CW)
            while fill:
                fill.pop(0)()

        def proj_block(sb):
            st = stg.tile([128, DIM], bf16, tag="st", name="st")
            wov = wo_sb[:].rearrange("p (o f) -> p o f", f=1024)
            for half in range(2):
                hs = slice(half * 512, (half + 1) * 512)
                pj = P["pp"].tile([128, 512], f32, tag="pp", name="pj")
                for kt in range(4):
                    nc.tensor.matmul(
                        pj[:],
                        OT[:, kt, sb * 128:(sb + 1) * 128],
                        wov[:, kt, hs],
                        start=(kt == 0), stop=(kt == 3))
                nc.vector.tensor_copy(st[:, hs], pj[:])
            # single whole-block output DMA: halves the sync-queue slots,
            # which also unblocks the next rep's input DMAs sooner
            nc.sync.dma_start(out.ap()[sb * 128:(sb + 1) * 128, :], st[:])

        # ---------------- emission schedule ----------------
        # Prologue: only what attention unit (ot0, cp0) reads -- K(0,0),
        # Q(0,0), V s2-blocks 0..3 -- at full acc-pipeline depth.
        with ExitStack() as s1:
            P["pp"] = s1.enter_context(
                tc.tile_pool(name="pp1", bufs=6, space="PSUM"))
            emit_kq(wkt, KT, 0, 0)
            emit_kq(wqt, QT, 0, 0)
            for st in range(4):
                emit_v_one(0, st)

        # Main pipeline: chunk-major over cp, pair units ot0..3, with
        # remaining projections / V blocks / output projections as PE
        # filler inside the units' j-loops.
        with ExitStack() as s2:
            P["pp"] = s2.enter_context(
                tc.tile_pool(name="pp2", bufs=2, space="PSUM"))
            P["psc"] = s2.enter_context(
                tc.tile_pool(name="psc", bufs=2, space="PSUM"))
            P["pso"] = s2.enter_context(
                tc.tile_pool(name="pso", bufs=1, space="PSUM"))

            def KQ(ot, sc):
                return [lambda: emit_kq(wkt, KT, ot, sc),
                        lambda: emit_kq(wqt, QT, ot, sc)]

            def V(sc, sts):
                return [lambda st=st: emit_v_one(sc, st) for st in sts]

            def PR(sbs):
                return [lambda sb=sb: proj_block(sb) for sb in sbs]

            wo_dma = [lambda: nc.gpsimd.dma_start(wo_sb[:], woT.ap())]

            # V blocks for phase cp+1 are emitted no later than unit (3, cp)
            # so PV reads never race their fills.
            # cp=0 phase (4-j units)
            emit_attn_pair(0, 0, fill=KQ(1, 0))
            emit_attn_pair(1, 0, fill=KQ(2, 0))
            emit_attn_pair(2, 0, fill=KQ(3, 0))
            emit_attn_pair(3, 0, fill=KQ(0, 1) + V(1, [0, 1, 2, 3]))
            # cp=1 phase (8-j units)
            emit_attn_pair(0, 1, fill=KQ(1, 1))
            emit_attn_pair(1, 1, fill=KQ(2, 1) + wo_dma)
            emit_attn_pair(2, 1, fill=KQ(3, 1) + PR([0]))
            emit_attn_pair(3, 1, fill=KQ(0, 2) + V(2, [0, 1, 2, 3]) + PR([1]))
            # cp=2 phase (12-j units)
            emit_attn_pair(0, 2, fill=KQ(1, 2) + PR([2]))
            emit_attn_pair(1, 2, fill=KQ(2, 2) + PR([3, 4]))
            emit_attn_pair(2, 2, fill=KQ(3, 2) + PR([5, 6]))
            emit_attn_pair(3, 2, fill=KQ(0, 3) + V(3, [0, 1, 2, 3]) + PR([7]))
            # cp=3 phase (16-j units)
            emit_attn_pair(0, 3, fill=KQ(1, 3) + PR([8]))
            emit_attn_pair(1, 3, fill=KQ(2, 3) + PR([9, 10]))
            emit_attn_pair(2, 3, fill=KQ(3, 3) + PR([11]))
            if EARLY_NORM:
                emit_attn_pair(3, 3, fill=PR([12, 13, 14, 15]),
                               fill_start=14, early_norm=True)
            else:
                emit_attn_pair(3, 3, fill=PR([12, 13, 14, 15]),
                               fill_start=99)


def _get_nc():
    global _NC
    if _NC is None:
        _NC = _build_nc()
    return _NC


def _retile_w(wt, o):
    # (o*128, f) -> (128, o*f): per-partition contiguous k-chunk-major
    f = wt.shape[1]
    return np.ascontiguousarray(
        wt.reshape(o, 128, f).transpose(1, 0, 2).reshape(128, o * f))


def _retile_w_otmajor(wt):
    # (1024, 512) -> (128, 4096) laid out [ot(4)][kt(8)][128]: pair ot's
    # weights contiguous in cols [ot*1024:(ot+1)*1024]
    return np.ascontiguousarray(
        wt.reshape(8, 128, 4, 128).transpose(1, 2, 0, 3).reshape(128, 4096))


def make_in_maps(x, Wq, Wk, Wv, Wo):
    cosT, sinT = _rope_tables()
    rT = _rot_matrix().astype(BF16)
    # keep where s2 <= s1 in (s2, s1) indexing -> upper-tri incl diag
    dm = np.triu(np.ones((128, 128), dtype=BF16))
    in_maps = []
    for c in range(N_CORES):
        b, g = c // 2, c % 2
        rows = slice(g * O, (g + 1) * O)
        xt = x[b].T.astype(BF16).reshape(8, 128, S)
        xtt = np.stack([
            np.ascontiguousarray(
                xt[:, :, sc * 512:(sc + 1) * 512]
            ).transpose(1, 0, 2).reshape(128, 4096)
            for sc in range(4)], axis=0).reshape(512, 4096)
        in_maps.append({
            "xTt": np.ascontiguousarray(xtt),
            "wqTt": _retile_w_otmajor(Wq[rows, :].T.astype(BF16)),
            "wkTt": _retile_w_otmajor(Wk[rows, :].T.astype(BF16)),
            "wvTt": _retile_w(Wv[rows, :].T.astype(BF16), 8),
            "woTt": _retile_w(Wo[:, rows].T.astype(BF16), 4),
            "cosT": cosT.astype(BF16), "sinT": sinT.astype(BF16),
            "rT": rT, "dmask": dm,
        })
    return in_maps


def _numpy_fallback(x, Wq, Wk, Wv, Wo, mask):
    cosT, sinT = _rope_tables()
    cos, sin = cosT[:64].T, sinT[:64].T                      # (S, 64)
    xq = x @ Wq.T
    xk = x @ Wk.T
    xv = x @ Wv.T

    def heads(t):
        return t.reshape(B, S, NUM_HEADS, HEAD_DIM).transpose(0, 2, 1, 3)

    q, k, v = heads(xq), heads(xk), heads(xv)

    def rot(t):
        return np.concatenate([-t[..., 32:], t[..., :32]], axis=-1)

    q = q * cos + rot(q) * sin
    k = k * cos + rot(k) * sin
    sc = np.einsum("bhsd,bhtd->bhst", q, k) / math.sqrt(HEAD_DIM)
    sc = np.where(mask[None, None] == 0, -np.inf, sc)
    sc = sc - sc.max(axis=-1, keepdims=True)
    e = np.exp(sc)
    p = e / e.sum(axis=-1, keepdims=True)
    o = np.einsum("bhst,bhtd->bhsd", p, v)
    o = o.transpose(0, 2, 1, 3).reshape(B, S, DIM)
    return (o @ Wo.T).astype(np.float32)


def kernel(x, Wq, Wk, Wv, Wo, mask):
    x = np.asarray(x)
    mask = np.asarray(mask)
    causal = bool(
        np.array_equal(np.asarray(mask, dtype=np.int64),
                       np.tril(np.ones((S, S), dtype=np.int64))))
    if not causal:
        return _numpy_fallback(
            np.asarray(x, np.float32), np.asarray(Wq, np.float32),
            np.asarray(Wk, np.float32), np.asarray(Wv, np.float32),
            np.asarray(Wo, np.float32), mask)

    from concourse.bass_utils import run_bass_kernel_spmd

    nc = _get_nc()
    in_maps = make_in_maps(x, Wq, Wk, Wv, Wo)
    res = run_bass_kernel_spmd(nc, in_maps, list(range(N_CORES)))
    out = np.empty((B, S, DIM), dtype=np.float32)
    for b in range(B):
        out[b] = (res.results[2 * b]["out"].astype(np.float32)
                  + res.results[2 * b + 1]["out"].astype(np.float32))
    return out


# revision 4
# speedup vs baseline: 1.6582x; 1.6582x over previous
"""Causal self-attention (RoPE) Trainium2 kernel, 8-core SPMD. v4.

Sharding: core c -> (batch b = c//2, head-group g = c%2). Each core computes
its batch's attention for its 8 heads, applies its 512 rows of Wo^T, and the
host sums the two head-group partials per batch (Megatron row-parallel).

v4 vs v3: the two heads sharing a 128-partition tile (hb=0 / hb=64) are
processed TOGETHER per (ot, chunk) unit. Their K=64 score matmuls are
emitted back-to-back so the PE runs them concurrently as (0,0)/(64,0)
row-tiles (tile_position auto-derived from base partitions; ~2x on the
score phase on HW - the cost model serializes them, hardware does not).
Score PSUM is one shared [128, 1024] tile (A cols 0:512, B 512:1024) so a
single wide 3D-AP exp covers both heads, halving activation fixed costs.
s1 is walked in 512-col chunks (4 per head); PSUM: 2x2 banks scores,
2x1 banks PV accumulators, 2 banks projection/rope accumulators.
"""

import math

import numpy as np
import ml_dtypes

BF16 = ml_dtypes.bfloat16

B, S, DIM = 4, 2048, 1024
NUM_HEADS = 16
HEAD_DIM = 64
ROPE_BASE = 10000.0
N_CORES = 8
HG = 8          # heads per core (head-group)
O = HG * HEAD_DIM  # 512 per-core projection width
CW = 512        # attention s1 chunk width

_NC = None  # cached compiled Bass program

# feature flags (HW-bringup bisection)
WIDE_EXP = True     # one 3D-AP exp covering both heads vs two 2D exps
EARLY_NORM = False  # early-norm cols [0:256] of unit (3,3) (split PV groups)


def _rope_tables():
    inv_freq = 1.0 / (ROPE_BASE ** (np.arange(0, HEAD_DIM, 2, dtype=np.float64) / HEAD_DIM))
    t = np.arange(S, dtype=np.float64)
    freqs = np.einsum("i,j->ij", t, inv_freq)          # (S, 32)
    emb = np.concatenate([freqs, freqs], axis=-1)      # (S, 64)
    cos = np.cos(emb).astype(np.float32)
    sin = np.sin(emb).astype(np.float32)
    # transposed + tiled to 128 partitions (2 heads per 128-row tile)
    cosT = np.tile(cos.T, (2, 1))                      # (128, S)
    sinT = np.tile(sin.T, (2, 1))
    return cosT, sinT


def _rot_matrix():
    # rotate_half as a matrix: out[d] = -q[d+32] (d<32), q[d-32] (d>=32)
    r = np.zeros((HEAD_DIM, HEAD_DIM), dtype=np.float32)
    for d in range(32):
        r[d, d + 32] = -1.0
        r[d + 32, d] = 1.0
    r128 = np.zeros((128, 128), dtype=np.float32)
    r128[:64, :64] = r
    r128[64:, 64:] = r
    return r128.T.copy()  # lhsT for out = R @ q


def _build_nc(reps=1):
    from contextlib import ExitStack

    import concourse.tile as tile
    from concourse import bacc
    import concourse.mybir as mybir

    f32 = mybir.dt.float32
    bf16 = mybir.dt.bfloat16

    nc = bacc.Bacc("TRN2", target_bir_lowering=False, debug=False,
                   num_devices=N_CORES)

    # host-retiled layouts: one contiguous 8KB run per partition per DMA
    xT = nc.declare_dram_parameter("xTt", [4 * 128, 4096], bf16,
                                   isOutput=False)
    wqT = nc.declare_dram_parameter("wqTt", [128, 4096], bf16, isOutput=False)
    wkT = nc.declare_dram_parameter("wkTt", [128, 4096], bf16, isOutput=False)
    wvT = nc.declare_dram_parameter("wvTt", [128, 4096], bf16, isOutput=False)
    woT = nc.declare_dram_parameter("woTt", [128, 4096], bf16, isOutput=False)
    cosT = nc.declare_dram_parameter("cosT", [128, S], bf16, isOutput=False)
    sinT = nc.declare_dram_parameter("sinT", [128, S], bf16, isOutput=False)
    rT = nc.declare_dram_parameter("rT", [128, 128], bf16, isOutput=False)
    dmask = nc.declare_dram_parameter("dmask", [128, 128], bf16, isOutput=False)
    out = nc.declare_dram_parameter("out", [S, DIM], bf16, isOutput=True)

    with tile.TileContext(nc) as tc, ExitStack() as top:
        for _ in range(reps):
            _emit_body(nc, tc, mybir, xT, wqT, wkT, wvT, woT,
                       cosT, sinT, rT, dmask, out)

    nc.compile()
    return nc


def _emit_body(nc, tc, mybir, xT, wqT, wkT, wvT, woT,
               cosT, sinT, rT, dmask, out):
    from contextlib import ExitStack

    f32 = mybir.dt.float32
    bf16 = mybir.dt.bfloat16

    xT3 = xT.ap().rearrange("(c p) f -> p c f", p=128)      # (128, 4, 4096)

    with ExitStack() as top:
        otp = top.enter_context(tc.tile_pool(name="otp", bufs=1))
        qk = top.enter_context(tc.tile_pool(name="qk", bufs=1))
        cst = top.enter_context(tc.tile_pool(name="cst", bufs=1))
        xp = top.enter_context(tc.tile_pool(name="xp", bufs=4))
        wp = top.enter_context(tc.tile_pool(name="wp", bufs=3))
        wop = top.enter_context(tc.tile_pool(name="wop", bufs=1))
        tp = top.enter_context(tc.tile_pool(name="tp", bufs=4))
        ep = top.enter_context(tc.tile_pool(name="ep", bufs=10))
        rp = top.enter_context(tc.tile_pool(name="rp", bufs=4))
        bp = top.enter_context(tc.tile_pool(name="bp", bufs=4))
        stg = top.enter_context(tc.tile_pool(name="stg", bufs=4))

        OT = otp.tile([128, 4, S], bf16)   # normalized attn out, transposed
        QT = qk.tile([128, 4, S], bf16)
        KT = qk.tile([128, 4, S], bf16)
        VA = qk.tile([128, 16, 520], bf16)  # [V(64) | ones] per head

        cos_sb = cst.tile([128, S], bf16)
        sin_sb = cst.tile([128, S], bf16)
        rt_sb = cst.tile([128, 128], bf16)
        dm_sb = cst.tile([128, 128], bf16)
        wo_sb = wop.tile([128, 4096], bf16)

        # ---------------- DMA issue (order = priority) ----------------
        # wq/wk are host-retiled ot-major: cols [ot*1024:(ot+1)*1024] hold
        # pair ot's 8 kt-chunks of 128, so the first unit's weights are one
        # contiguous quarter. x chunks are kt-major (any emit reads it all);
        # quarter-split x0 so the prologue's kt-sequential reads start early.
        wk_sb = wp.tile([128, 4096], bf16, tag="w", name="wk_sb")
        wq_sb = wp.tile([128, 4096], bf16, tag="w", name="wq_sb")
        wv_sb = wp.tile([128, 4096], bf16, tag="w", name="wv_sb")
        xs = [xp.tile([128, 4096], bf16, tag="x", name=f"x{sc}")
              for sc in range(4)]
        nc.sync.dma_start(wk_sb[:, 0:1024], wkT.ap()[:, 0:1024])
        for q in range(4):
            nc.sync.dma_start(xs[0][:, q * 1024:(q + 1) * 1024],
                              xT3[:, 0, q * 1024:(q + 1) * 1024])
        nc.sync.dma_start(wq_sb[:, 0:1024], wqT.ap()[:, 0:1024])
        nc.sync.dma_start(wv_sb[:, 0:2048], wvT.ap()[:, 0:2048])
        nc.sync.dma_start(wv_sb[:, 2048:4096], wvT.ap()[:, 2048:4096])
        nc.sync.dma_start(wk_sb[:, 1024:2048], wkT.ap()[:, 1024:2048])
        nc.sync.dma_start(wq_sb[:, 1024:2048], wqT.ap()[:, 1024:2048])
        nc.sync.dma_start(wk_sb[:, 2048:4096], wkT.ap()[:, 2048:4096])
        nc.sync.dma_start(wq_sb[:, 2048:4096], wqT.ap()[:, 2048:4096])
        nc.sync.dma_start(xs[1][:, 0:2048], xT3[:, 1, 0:2048])
        nc.sync.dma_start(xs[1][:, 2048:4096], xT3[:, 1, 2048:4096])
        for sc in range(2, 4):
            nc.sync.dma_start(xs[sc][:], xT3[:, sc, :])
        nc.gpsimd.dma_start(cos_sb[:, 0:1024], cosT.ap()[:, 0:1024])
        nc.gpsimd.dma_start(sin_sb[:, 0:1024], sinT.ap()[:, 0:1024])
        nc.gpsimd.dma_start(rt_sb[:], rT.ap())
        nc.gpsimd.dma_start(dm_sb[:], dmask.ap())
        nc.gpsimd.dma_start(cos_sb[:, 1024:2048], cosT.ap()[:, 1024:2048])
        nc.gpsimd.dma_start(sin_sb[:, 1024:2048], sinT.ap()[:, 1024:2048])

        # ot-major views for q/k; kt-major full-width for v
        wkt = wk_sb[:].rearrange("p (o k f) -> p o k f", o=4, k=8)
        wqt = wq_sb[:].rearrange("p (o k f) -> p o k f", o=4, k=8)
        wvt = wv_sb[:].rearrange("p (o f) -> p o f", f=512)

        def xtile(sc, kt, csl):
            xv = xs[sc][:].rearrange("p (o s) -> p o s", s=512)
            return xv[:, kt, csl]

        P = {}  # current-scope PSUM pools: "pp", "psc", "pso"

        def rope(acc, dest, ot, sl):
            # dest[:, ot, sl] = acc*cos + R @ (acc*sin)
            rs = tp.tile([128, 512], bf16, tag="t", name="rs")
            nc.vector.tensor_mul(rs[:], acc[:], sin_sb[:, sl])
            rot = P["pp"].tile([128, 512], f32, tag="pp", name="rot")
            nc.tensor.matmul(rot[:], rt_sb[:], rs[:], start=True, stop=True)
            t1 = tp.tile([128, 512], f32, tag="t", name="t1")
            nc.vector.tensor_mul(t1[:], acc[:], cos_sb[:, sl])
            nc.vector.tensor_add(dest[:, ot, sl], t1[:], rot[:])

        def emit_kq(wlist, dest, ot, sc):
            sl = slice(sc * 512, (sc + 1) * 512)
            acc = P["pp"].tile([128, 512], f32, tag="pp", name="acc")
            for kt in range(8):
                nc.tensor.matmul(
                    acc[:],
                    wlist[:, ot, kt, :],
                    xtile(sc, kt, slice(0, 512)),
                    start=(kt == 0), stop=(kt == 7))
            rope(acc, dest, ot, sl)

        def emit_v_one(sc, st):
            s2t = sc * 4 + st
            acc = P["pp"].tile([128, 512], f32, tag="pp", name="acc")
            for kt in range(8):
                nc.tensor.matmul(
                    acc[:],
                    xtile(sc, kt, slice(st * 128, (st + 1) * 128)),
                    wvt[:, kt, :],
                    start=(kt == 0), stop=(kt == 7))
            vsl = VA[:, s2t, :].rearrange("p (h c) -> p h c", c=65)
            nc.scalar.copy(
                vsl[:, :, 0:64],
                acc[:].rearrange("p (h c) -> p h c", c=64))
            nc.gpsimd.memset(vsl[:, :, 64:65], 1.0)

        def emit_attn_pair(ot, cp, fill=None, fill_start=0, early_norm=False):
            # both heads of pair ot; s1 chunk cp (512 wide).
            # fill: list of zero-arg callables, one consumed per j.
            # early_norm: normalize cols [0:256] as soon as their last PV
            # lands (j == 4cp+1) so proj fills of this chunk can run mid-unit.
            njs = 4 * cp + 4
            hA, hB = 2 * ot, 2 * ot + 1
            s1l = cp * CW
            otA = P["pso"].tile([65, CW], f32, tag="oA", name="otA")
            otB = P["pso"].tile([65, CW], f32, tag="oB", name="otB")
            # fill pops are spread every 4th j (end-anchored) and happen
            # BEFORE the pending PV so the fill covers its exp wait;
            # leftovers drain after the norms, covering the next unit's
            # first-PV wait on this unit's pso slots.
            def pops_now(j):
                return (fill and j >= fill_start
                        and (njs - 1 - j) % 3 == 0)

            def emit_pv(j, l0, et):
                # (dst-lo, dst-hi, start, stop) col spans; with early_norm
                # cols [0:256] form their own accumulation group closed at
                # j == 4cp+1 so the early norm may read them mid-unit.
                if early_norm and j <= 4 * cp + 1:
                    # separate spans so cols [0:256] take no further writes
                    # after j == 4cp+1; start=True only on j=0's first span
                    # (it marks the whole 2KB zero region pending-zero), and
                    # skip the per-bank group check since the early norm
                    # reads mid-group (writes are col-disjoint by then).
                    spans = [(l0, 256, j == 0), (256, CW, False)]
                else:
                    spans = [(l0, CW, j == 0)]
                skip = early_norm
                for lo, hi, st in spans:
                    nc.tensor.matmul(
                        otA[:, lo:hi],
                        VA[:, j, hA * 65:(hA + 1) * 65],
                        et[:, lo:hi],
                        start=st, stop=(j == njs - 1),
                        skip_group_check=skip)
                    nc.tensor.matmul(
                        otB[:, lo:hi],
                        VA[:, j, hB * 65:(hB + 1) * 65],
                        et[:, CW + lo:CW + hi],
                        start=st, stop=(j == njs - 1),
                        skip_group_check=skip)

            def norm(otps, hb, lo, hi):
                # GPSIMD cannot touch PSUM, so the otps reads (recip + mul)
                # must stay on DVE; only the broadcast runs on Pool
                w = hi - lo
                rec = rp.tile([1, CW], f32, tag="rec", name="rec")
                nc.vector.reciprocal(rec[:, 0:w], otps[64:65, lo:hi])
                bc = bp.tile([64, CW], f32, tag="bc", name="bc")
                nc.gpsimd.partition_broadcast(bc[:, 0:w], rec[:, 0:w])
                nc.vector.tensor_mul(
                    OT[hb:hb + 64, ot, s1l + lo:s1l + hi],
                    otps[0:64, lo:hi], bc[:, 0:w])

            pending = None
            for j in range(njs):
                l0 = max(0, 128 * j - s1l)
                scp = P["psc"].tile([128, 2 * CW], f32, tag="sc", name="scp")
                nc.tensor.matmul(
                    scp[:, l0:CW],
                    KT[0:64, ot, j * 128:(j + 1) * 128],
                    QT[0:64, ot, s1l + l0:s1l + CW],
                    start=True, stop=True)
                nc.tensor.matmul(
                    scp[:, CW + l0:2 * CW],
                    KT[64:128, ot, j * 128:(j + 1) * 128],
                    QT[64:128, ot, s1l + l0:s1l + CW],
                    start=True, stop=True)
                et = ep.tile([128, 2 * CW], bf16, tag="e", name="et")
                if WIDE_EXP:
                    a_in = scp[:].rearrange("p (b w) -> p b w", b=2)[:, :, l0:CW]
                    a_out = et[:].rearrange("p (b w) -> p b w", b=2)[:, :, l0:CW]
                    nc.scalar.activation(
                        a_out, a_in,
                        mybir.ActivationFunctionType.Exp,
                        scale=1.0 / math.sqrt(HEAD_DIM))
                else:
                    nc.scalar.activation(
                        et[:, l0:CW], scp[:, l0:CW],
                        mybir.ActivationFunctionType.Exp,
                        scale=1.0 / math.sqrt(HEAD_DIM))
                    nc.scalar.activation(
                        et[:, CW + l0:2 * CW], scp[:, CW + l0:2 * CW],
                        mybir.ActivationFunctionType.Exp,
                        scale=1.0 / math.sqrt(HEAD_DIM))
                if 128 * j >= s1l:
                    dl = 128 * j - s1l
                    nc.gpsimd.tensor_mul(
                        et[:, dl:dl + 128], et[:, dl:dl + 128], dm_sb[:])
                    nc.gpsimd.tensor_mul(
                        et[:, CW + dl:CW + dl + 128],
                        et[:, CW + dl:CW + dl + 128], dm_sb[:])
                if pops_now(j):
                    fill.pop(0)()
                if pending is not None:
                    emit_pv(*pending)
                    if early_norm and pending[0] == 4 * cp + 1:
                        norm(otA, 0, 0, 256)
                        norm(otB, 64, 0, 256)
                pending = (j, l0, et)
            emit_pv(*pending)
            lo = 256 if early_norm else 0
            norm(otA, 0, lo, CW)
            norm(otB, 64, lo, CW)
            while fill:
                fill.pop(0)()

        def proj_block(sb):
            st = stg.tile([128, DIM], bf16, tag="st", name="st")
            wov = wo_sb[:].rearrange("p (o f) -> p o f", f=1024)
            for half in range(2):
                hs = slice(half * 512, (half + 1) * 512)
                pj = P["pp"].tile([128, 512], f32, tag="pp", name="pj")
                for kt in range(4):
                    nc.tensor.matmul(
                        pj[:],
                        OT[:, kt, sb * 128:(sb + 1) * 128],
                        wov[:, kt, hs],
                        start=(kt == 0), stop=(kt == 3))
                nc.vector.tensor_copy(st[:, hs], pj[:])
            # single whole-block output DMA: halves the sync-queue slots,
            # which also unblocks the next rep's input DMAs sooner
            nc.sync.dma_start(out.ap()[sb * 128:(sb + 1) * 128, :], st[:])

        # ---------------- emission schedule ----------------
        # Prologue: only what attention unit (ot0, cp0) reads -- K(0,0),
        # Q(0,0), V s2-blocks 0..3 -- at full acc-pipeline depth.
        with ExitStack() as s1:
            P["pp"] = s1.enter_context(
                tc.tile_pool(name="pp1", bufs=6, space="PSUM"))
            emit_kq(wkt, KT, 0, 0)
            emit_kq(wqt, QT, 0, 0)
            for st in range(4):
                emit_v_one(0, st)

        # Main pipeline: chunk-major over cp, pair units ot0..3, with
        # remaining projections / V blocks / output projections as PE
        # filler inside the units' j-loops.
        with ExitStack() as s2:
            P["pp"] = s2.enter_context(
                tc.tile_pool(name="pp2", bufs=2, space="PSUM"))
            P["psc"] = s2.enter_context(
                tc.tile_pool(name="psc", bufs=2, space="PSUM"))
            P["pso"] = s2.enter_context(
                tc.tile_pool(name="pso", bufs=1, space="PSUM"))

            def KQ(ot, sc):
                return [lambda: emit_kq(wkt, KT, ot, sc),
                        lambda: emit_kq(wqt, QT, ot, sc)]

            def V(sc, sts):
                return [lambda st=st: emit_v_one(sc, st) for st in sts]

            def PR(sbs):
                return [lambda sb=sb: proj_block(sb) for sb in sbs]

            wo_dma = [lambda: nc.gpsimd.dma_start(wo_sb[:], woT.ap())]

            # V blocks for phase cp+1 are emitted no later than unit (3, cp)
            # so PV reads never race their fills.
            # cp=0 phase (4-j units)
            emit_attn_pair(0, 0, fill=KQ(1, 0))
            emit_attn_pair(1, 0, fill=KQ(2, 0))
            emit_attn_pair(2, 0, fill=KQ(3, 0))
            emit_attn_pair(3, 0, fill=KQ(0, 1) + V(1, [0, 1, 2, 3]))
            # cp=1 phase (8-j units)
            emit_attn_pair(0, 1, fill=KQ(1, 1))
            emit_attn_pair(1, 1, fill=KQ(2, 1) + wo_dma)
            emit_attn_pair(2, 1, fill=KQ(3, 1) + PR([0]))
            emit_attn_pair(3, 1, fill=KQ(0, 2) + V(2, [0, 1, 2, 3]) + PR([1]))
            # cp=2 phase (12-j units)
            emit_attn_pair(0, 2, fill=KQ(1, 2) + PR([2]))
            emit_attn_pair(1, 2, fill=KQ(2, 2) + PR([3, 4]))
            emit_attn_pair(2, 2, fill=KQ(3, 2) + PR([5, 6]))
            emit_attn_pair(3, 2, fill=KQ(0, 3) + V(3, [0, 1, 2, 3]) + PR([7]))
            # cp=3 phase (16-j units)
            emit_attn_pair(0, 3, fill=KQ(1, 3) + PR([8]))
            emit_attn_pair(1, 3, fill=KQ(2, 3) + PR([9, 10]))
            emit_attn_pair(2, 3, fill=KQ(3, 3) + PR([11]))
            if EARLY_NORM:
                emit_attn_pair(3, 3, fill=PR([12, 13, 14, 15]),
                               fill_start=14, early_norm=True)
            else:
                emit_attn_pair(3, 3, fill=PR([12, 13, 14, 15]),
                               fill_start=99)


def _get_nc():
    global _NC
    if _NC is None:
        _NC = _build_nc()
    return _NC


def _retile_w(wt, o):
    # (o*128, f) -> (128, o*f): per-partition contiguous k-chunk-major
    f = wt.shape[1]
    return np.ascontiguousarray(
        wt.reshape(o, 128, f).transpose(1, 0, 2).reshape(128, o * f))


def _retile_w_otmajor(wt):
    # (1024, 512) -> (128, 4096) laid out [ot(4)][kt(8)][128]: pair ot's
    # weights contiguous in cols [ot*1024:(ot+1)*1024]
    return np.ascontiguousarray(
        wt.reshape(8, 128, 4, 128).transpose(1, 2, 0, 3).reshape(128, 4096))


def make_in_maps(x, Wq, Wk, Wv, Wo):
    cosT, sinT = _rope_tables()
    rT = _rot_matrix().astype(BF16)
    # keep where s2 <= s1 in (s2, s1) indexing -> upper-tri incl diag
    dm = np.triu(np.ones((128, 128), dtype=BF16))
    in_maps = []
    for c in range(N_CORES):
        b, g = c // 2, c % 2
        rows = slice(g * O, (g + 1) * O)
        xt = x[b].T.astype(BF16).reshape(8, 128, S)
        xtt = np.stack([
            np.ascontiguousarray(
                xt[:, :, sc * 512:(sc + 1) * 512]
            ).transpose(1, 0, 2).reshape(128, 4096)
            for sc in range(4)], axis=0).reshape(512, 4096)
        in_maps.append({
            "xTt": np.ascontiguousarray(xtt),
            "wqTt": _retile_w_otmajor(Wq[rows, :].T.astype(BF16)),
            "wkTt": _retile_w_otmajor(Wk[rows, :].T.astype(BF16)),
            "wvTt": _retile_w(Wv[rows, :].T.astype(BF16), 8),
            "woTt": _retile_w(Wo[:, rows].T.astype(BF16), 4),
            "cosT": cosT.astype(BF16), "sinT": sinT.astype(BF16),
            "rT": rT, "dmask": dm,
        })
    return in_maps


def _numpy_fallback(x, Wq, Wk, Wv, Wo, mask):
    cosT, sinT = _rope_tables()
    cos, sin = cosT[:64].T, sinT[:64].T                      # (S, 64)
    xq = x @ Wq.T
    xk = x @ Wk.T
    xv = x @ Wv.T

    def heads(t):
        return t.reshape(B, S, NUM_HEADS, HEAD_DIM).transpose(0, 2, 1, 3)

    q, k, v = heads(xq), heads(xk), heads(xv)

    def rot(t):
        return np.concatenate([-t[..., 32:], t[..., :32]], axis=-1)

    q = q * cos + rot(q) * sin
    k = k * cos + rot(k) * sin
    sc = np.einsum("bhsd,bhtd->bhst", q, k) / math.sqrt(HEAD_DIM)
    sc = np.where(mask[None, None] == 0, -np.inf, sc)
    sc = sc - sc.max(axis=-1, keepdims=True)
    e = np.exp(sc)
    p = e / e.sum(axis=-1, keepdims=True)
    o = np.einsum("bhst,bhtd->bhsd", p, v)
    o = o.transpose(0, 2, 1, 3).reshape(B, S, DIM)
    return (o @ Wo.T).astype(np.float32)


def kernel(x, Wq, Wk, Wv, Wo, mask):
    x = np.asarray(x)
    mask = np.asarray(mask)
    causal = bool(
        np.array_equal(np.asarray(mask, dtype=np.int64),
                       np.tril(np.ones((S, S), dtype=np.int64))))
    if not causal:
        return _numpy_fallback(
            np.asarray(x, np.float32), np.asarray(Wq, np.float32),
            np.asarray(Wk, np.float32), np.asarray(Wv, np.float32),
            np.asarray(Wo, np.float32), mask)

    from concourse.bass_utils import run_bass_kernel_spmd

    nc = _get_nc()
    in_maps = make_in_maps(x, Wq, Wk, Wv, Wo)
    res = run_bass_kernel_spmd(nc, in_maps, list(range(N_CORES)))
    out = np.empty((B, S, DIM), dtype=np.float32)
    for b in range(B):
        out[b] = (res.results[2 * b]["out"].astype(np.float32)
                  + res.results[2 * b + 1]["out"].astype(np.float32))
    return out
